# revision 13
# baseline (speedup 1.0000x reference)
"""Trainium2 Bass kernel for the ADMG RKHS-DAGMA gradient contraction.

Reference computation (D=8 variables, N=1500 observations):
    output[i, j] = sum_l alpha[j, l] * K[j, i, l]
                 + sum_{a, l} beta[j, a, l] * grad_K2[j, i, l, a]     [N, D]
    Sigma = L @ L.T + 1e-6 * I                                        [D, D]

Sharding: variable-parallel over the leading d axis — core j owns K[j]
(9 MB) and grad_K2[j] (72 MB) and produces output column j. No
collectives are needed; columns are gathered on the host.

Per-core kernel: a weighted row-sum. With G = grad_K2[j] viewed as
[N, L*A] = [1500, 12000] and vrow = concat(beta[j].T.flat, alpha[j])
(13500 f32), the output column is
    out[i] = sum_m [G | K][i, m] * vrow[m].
The multiplier vector is pre-broadcast across 128 partitions and each
[128, C] data tile goes through one fused DVE tensor_tensor_reduce
(multiply + free-axis reduce), so every streamed element passes through
the VectorEngine exactly once (~165us) and the kernel stays DMA-bound
(~81 MB/core at ~360 GB/s).
"""

import numpy as np

D = 8
N = 1500
NCORES = 8
MG = D * N          # 12000: grad_K2 inner (l, a) length
MTOT = MG + N       # 13500: plus K's l axis
P = 128
NT = (N + P - 1) // P   # 12 i-tiles (last one 92 rows)
GC = 6000               # g is streamed in chunks of this many columns
NGC = MG // GC          # 2 chunks

_COMPILED = None


def _build():
    from concourse import bacc, mybir
    from concourse.tile import TileContext

    f32 = mybir.dt.float32
    nc = bacc.Bacc()

    g = nc.declare_dram_parameter("g", [N, MG], f32, isOutput=False)
    k = nc.declare_dram_parameter("k", [N, N], f32, isOutput=False)
    vb = nc.declare_dram_parameter("vb", [P, MTOT], f32, isOutput=False)
    # Augmented [2D, D]: rows 0..D-1 = L.T, rows D..2D-1 = 1e-3 * I, so that
    # ltaug.T @ ltaug = L @ L.T + 1e-6 * I in a single matmul (the TRN2
    # instruction encoding has one wait slot, so a separate +eye TensorTensor
    # with two upstream deps fails walrus codegen).
    lt = nc.declare_dram_parameter("lt", [2 * D, D], f32, isOutput=False)
    o = nc.declare_dram_parameter("o", [P, NT], f32, isOutput=True)
    sig = nc.declare_dram_parameter("sig", [D, D], f32, isOutput=True)

    mult = mybir.AluOpType.mult
    add = mybir.AluOpType.add

    with TileContext(nc) as tc:
        with (
            tc.tile_pool(name="const", bufs=1) as cpool,
            tc.tile_pool(name="gdata", bufs=3) as gpool,
            tc.tile_pool(name="kdata", bufs=2) as kpool,
            tc.tile_pool(name="accs", bufs=8) as apool,
            tc.tile_pool(name="psum", bufs=1, space="PSUM") as ppool,
        ):
            # Multiplier vector, pre-replicated across partitions on host.
            vb_sb = cpool.tile([P, MTOT], f32)
            nc.sync.dma_start(out=vb_sb[:, :], in_=vb[:, :])

            # Prime DVE's vector clock on the vb DMA so none of the later
            # tensor_tensor_reduce ops needs a second semaphore wait for it.
            # (Every instruction except EventSemaphore encodes exactly one
            # semaphore wait on TRN2 — two waits fail walrus codegen.)
            primer = cpool.tile([P, 1], f32)
            nc.vector.tensor_copy(out=primer[:1, :], in_=vb_sb[:1, :1])

            # Sigma = ltaug.T @ ltaug on the (otherwise idle) TensorEngine.
            lt_sb = cpool.tile([2 * D, D], f32)
            nc.sync.dma_start(out=lt_sb[:, :], in_=lt[:, :])
            sig_ps = ppool.tile([D, D], f32)
            nc.tensor.matmul(sig_ps[:, :], lt_sb[:, :], lt_sb[:, :],
                             start=True, stop=True)
            sig_sb = cpool.tile([D, D], f32)
            nc.vector.tensor_copy(out=sig_sb[:, :], in_=sig_ps[:, :])
            nc.sync.dma_start(out=sig[:, :], in_=sig_sb[:, :])

            out_all = cpool.tile([P, NT], f32)
            nc.vector.memset(out_all[:, :], 0.0)

            for t in range(NT):
                p = min(P, N - t * P)
                i0 = t * P
                # Each TTR gets a private [P, 2] tile: column 0 holds the
                # partial sum, column 1 (stride-0 broadcast) absorbs the dead
                # full-size `out`. Private tiles keep the DVE ops dependency-
                # free among themselves so each carries exactly one wait (its
                # input DMA).
                accs = []
                for c in range(NGC):
                    gt = gpool.tile([P, GC], f32, tag="g")
                    nc.sync.dma_start(out=gt[:p, :],
                                      in_=g[i0:i0 + p, c * GC:(c + 1) * GC])
                    acc = apool.tile([P, 2], f32, tag="acc")
                    # Fused multiply + free-axis reduce in one DVE pass:
                    # out = (gt * 1.0) * vb, accum = row-sum(out). The raw-ISA
                    # tensor_tensor_reduce encoding crashes the exec unit on
                    # this runtime; InstTensorScalarPtr does the same thing.
                    nc.vector.scalar_tensor_tensor(
                        out=acc[:p, 1:2].broadcast_to((p, GC)),
                        in0=gt[:p, :],
                        scalar=1.0,
                        in1=vb_sb[:p, c * GC:(c + 1) * GC],
                        op0=mult,
                        op1=mult,
                        accum_out=acc[:p, 0:1],
                    )
                    accs.append(acc)
                kt = kpool.tile([P, N], f32, tag="k")
                nc.sync.dma_start(out=kt[:p, :], in_=k[i0:i0 + p, :])
                acc = apool.tile([P, 2], f32, tag="acc")
                nc.vector.scalar_tensor_tensor(
                    out=acc[:p, 1:2].broadcast_to((p, N)),
                    in0=kt[:p, :],
                    scalar=1.0,
                    in1=vb_sb[:p, MG:MTOT],
                    op0=mult,
                    op1=mult,
                    accum_out=acc[:p, 0:1],
                )
                accs.append(acc)
                # out[:, t] = acc0 + acc1 + acc2 in one op (two tensor inputs
                # plus a per-partition scalar AP).
                nc.vector.scalar_tensor_tensor(
                    out=out_all[:p, t:t + 1],
                    in0=accs[0][:p, 0:1],
                    scalar=accs[1][:p, 0:1],
                    in1=accs[2][:p, 0:1],
                    op0=add,
                    op1=add,
                )
            nc.sync.dma_start(out=o[:, :], in_=out_all[:, :])
    nc.finalize()
    return nc


def _get_nc():
    global _COMPILED
    if _COMPILED is None:
        _COMPILED = _build()
    return _COMPILED


def run(inputs, trace=False):
    """Run the SPMD kernel; returns ((output, Sigma), BassKernelResults)."""
    from concourse.bass_utils import run_bass_kernel_spmd

    alpha = np.ascontiguousarray(np.asarray(inputs["alpha"], dtype=np.float32))
    beta = np.ascontiguousarray(np.asarray(inputs["beta"], dtype=np.float32))
    L = np.ascontiguousarray(np.asarray(inputs["L"], dtype=np.float32))
    K = np.ascontiguousarray(np.asarray(inputs["K"], dtype=np.float32))
    grad_K2 = np.ascontiguousarray(np.asarray(inputs["grad_K2"], dtype=np.float32))

    ltaug = np.concatenate(
        [L.T, 1e-3 * np.eye(D, dtype=np.float32)], axis=0
    ).astype(np.float32)

    in_maps = []
    for j in range(NCORES):
        vrow = np.empty(MTOT, dtype=np.float32)
        vrow[:MG] = np.ascontiguousarray(beta[j].T).reshape(-1)
        vrow[MG:] = alpha[j]
        in_maps.append({
            "g": grad_K2[j].reshape(N, MG),
            "k": K[j],
            "vb": np.ascontiguousarray(np.broadcast_to(vrow, (P, MTOT))),
            "lt": ltaug,
        })

    nc = _get_nc()
    res = run_bass_kernel_spmd(nc, in_maps, core_ids=list(range(NCORES)),
                               trace=trace)
    output = np.empty((N, D), dtype=np.float32)
    for j in range(NCORES):
        col = res.results[j]["o"]          # [128, 12]
        output[:, j] = col.T.reshape(-1)[:N]
    Sigma = res.results[0]["sig"]
    return (output, Sigma), res


def kernel(**inputs):
    out, _ = run(inputs)
    return out


# revision 19
# speedup vs baseline: 1.0006x; 1.0006x over previous
"""Trainium2 Bass kernel for the ADMG RKHS-DAGMA gradient contraction.

Reference computation (D=8 variables, N=1500 observations):
    output[i, j] = sum_l alpha[j, l] * K[j, i, l]
                 + sum_{a, l} beta[j, a, l] * grad_K2[j, i, l, a]     [N, D]
    Sigma = L @ L.T + 1e-6 * I                                        [D, D]

Sharding: variable-parallel over the leading d axis — core j owns K[j]
(9 MB) and grad_K2[j] (72 MB) and produces output column j. No
collectives are needed; columns are gathered on the host.

Per-core kernel: a weighted row-sum. With G = grad_K2[j] viewed as
[N, L*A] = [1500, 12000] and vrow = concat(beta[j].T.flat, alpha[j])
(13500 f32), the output column is
    out[i] = sum_m [G | K][i, m] * vrow[m].
vrow is broadcast across the 128 partitions on-chip (PE ones-matmul into
PSUM, ScalarE drains to SBUF), then every streamed [128, C] tile goes
through one fused DVE scalar_tensor_tensor (multiply + free-axis
accumulate), so each element passes the VectorEngine exactly once
(~170us) and the kernel stays DMA-bound (~82 MB/core at ~360 GB/s).

TRN2 toolchain constraints baked in here:
  - every instruction except EventSemaphore encodes exactly ONE
    semaphore wait; Tile emits inline waits, so the kernel is structured
    (private acc tiles, clock-priming reads) to never need two;
  - the raw-ISA tensor_tensor_reduce encoding crashes the exec unit on
    this runtime; InstTensorScalarPtr (scalar_tensor_tensor with
    accum_out) performs the same fused multiply+reduce;
  - Sigma's +1e-6*I rides inside the matmul via an augmented-contraction
    operand [L.T ; 1e-3*I] so no PSUM+SBUF add is needed.
"""

import numpy as np

D = 8
N = 1500
NCORES = 8
MG = D * N            # 12000: grad_K2 inner (l, a) length
MTOT = MG + N         # 13500: plus K's l axis
P = 128
NT = (N + P - 1) // P  # 12 i-tiles (last one 92 rows)
BC = 512               # broadcast chunk (one PSUM bank of f32)
NBC = (MTOT + BC - 1) // BC          # 27 chunks
MPAD = NBC * BC                      # 13824
VHLEN = MPAD + P                     # vrow (padded) + a ones-vector tail

_COMPILED = None


def _build():
    from concourse import bacc, mybir
    from concourse.tile import TileContext

    f32 = mybir.dt.float32
    nc = bacc.Bacc()

    g = nc.declare_dram_parameter("g", [N, MG], f32, isOutput=False)
    k = nc.declare_dram_parameter("k", [N, N], f32, isOutput=False)
    # One row: vrow zero-padded to MPAD, then 128 ones (the PE broadcast
    # lhsT) — a single row so both matmul operands sit at base partition 0
    # and depend on one DMA.
    vh = nc.declare_dram_parameter("vh", [1, VHLEN], f32, isOutput=False)
    # Rows 0..D-1 = L.T, rows D..2D-1 = 1e-3 * I, so that one matmul gives
    # ltaug.T @ ltaug = L @ L.T + 1e-6 * I.
    lt = nc.declare_dram_parameter("lt", [2 * D, D], f32, isOutput=False)
    o = nc.declare_dram_parameter("o", [P, NT], f32, isOutput=True)
    sig = nc.declare_dram_parameter("sig", [D, D], f32, isOutput=True)

    mult = mybir.AluOpType.mult
    add = mybir.AluOpType.add

    with TileContext(nc) as tc:
        with (
            tc.tile_pool(name="const", bufs=1) as cpool,
            tc.tile_pool(name="gdata", bufs=2) as gpool,
            tc.tile_pool(name="kdata", bufs=2) as kpool,
            tc.tile_pool(name="accs", bufs=8) as apool,
            tc.tile_pool(name="psum", bufs=4, space="PSUM") as ppool,
            tc.tile_pool(name="psig", bufs=1, space="PSUM") as pspool,
        ):
            vh_sb = cpool.tile([1, VHLEN], f32)
            nc.sync.dma_start(out=vh_sb[:, :], in_=vh[:, :])

            # Broadcast vrow across partitions: out[p, f] = ones[0, p] *
            # vrow[0, f] per 512-wide chunk; ScalarE (idle otherwise) drains
            # PSUM to SBUF.
            vb_sb = cpool.tile([P, MPAD], f32)
            for c in range(NBC):
                ps = ppool.tile([P, BC], f32, tag="bc")
                nc.tensor.matmul(ps[:, :], vh_sb[0:1, MPAD:MPAD + P],
                                 vh_sb[0:1, c * BC:(c + 1) * BC],
                                 start=True, stop=True)
                nc.scalar.copy(out=vb_sb[:, c * BC:(c + 1) * BC], in_=ps[:, :])

            # Prime DVE's vector clock on the last broadcast copy so the
            # compute ops below need no second wait for vb_sb (ScalarE is
            # serial, so observing copy #26 covers all of them).
            primer = cpool.tile([P, 1], f32)
            nc.vector.tensor_copy(out=primer[:1, :],
                                  in_=vb_sb[:1, MPAD - 1:MPAD])

            # Sigma = ltaug.T @ ltaug on the TensorEngine.
            lt_sb = cpool.tile([2 * D, D], f32)
            nc.sync.dma_start(out=lt_sb[:, :], in_=lt[:, :])
            sig_ps = pspool.tile([D, D], f32)
            nc.tensor.matmul(sig_ps[:, :], lt_sb[:, :], lt_sb[:, :],
                             start=True, stop=True)
            sig_sb = cpool.tile([D, D], f32)
            nc.vector.tensor_copy(out=sig_sb[:, :], in_=sig_ps[:, :])
            nc.sync.dma_start(out=sig[:, :], in_=sig_sb[:, :])

            out_all = cpool.tile([P, NT], f32)
            nc.vector.memset(out_all[:, :], 0.0)

            GC = MG // 2   # 6000-wide g chunks (two per i-tile) to fit SBUF
            for t in range(NT):
                p = min(P, N - t * P)
                i0 = t * P
                # Fused multiply+reduce per streamed tile: out = (data * 1.0)
                # * vb, accum = row-sum. Column 1 of each private acc tile
                # absorbs the dead full-size out via a stride-0 broadcast AP.
                accs = []
                for c in range(2):
                    gt = gpool.tile([P, GC], f32, tag="g")
                    nc.sync.dma_start(out=gt[:p, :],
                                      in_=g[i0:i0 + p, c * GC:(c + 1) * GC])
                    acc = apool.tile([P, 2], f32, tag="acc")
                    nc.vector.scalar_tensor_tensor(
                        out=acc[:p, 1:2].broadcast_to((p, GC)),
                        in0=gt[:p, :],
                        scalar=1.0,
                        in1=vb_sb[:p, c * GC:(c + 1) * GC],
                        op0=mult,
                        op1=mult,
                        accum_out=acc[:p, 0:1],
                    )
                    accs.append(acc)
                kt = kpool.tile([P, N], f32, tag="k")
                nc.sync.dma_start(out=kt[:p, :], in_=k[i0:i0 + p, :])
                acc = apool.tile([P, 2], f32, tag="acc")
                nc.vector.scalar_tensor_tensor(
                    out=acc[:p, 1:2].broadcast_to((p, N)),
                    in0=kt[:p, :],
                    scalar=1.0,
                    in1=vb_sb[:p, MG:MTOT],
                    op0=mult,
                    op1=mult,
                    accum_out=acc[:p, 0:1],
                )
                accs.append(acc)
                nc.vector.scalar_tensor_tensor(
                    out=out_all[:p, t:t + 1],
                    in0=accs[0][:p, 0:1],
                    scalar=accs[1][:p, 0:1],
                    in1=accs[2][:p, 0:1],
                    op0=add,
                    op1=add,
                )
            nc.sync.dma_start(out=o[:, :], in_=out_all[:, :])
    nc.finalize()
    return nc


def _get_nc():
    global _COMPILED
    if _COMPILED is None:
        _COMPILED = _build()
    return _COMPILED


def run(inputs, trace=False):
    """Run the SPMD kernel; returns ((output, Sigma), BassKernelResults)."""
    from concourse.bass_utils import run_bass_kernel_spmd

    alpha = np.ascontiguousarray(np.asarray(inputs["alpha"], dtype=np.float32))
    beta = np.ascontiguousarray(np.asarray(inputs["beta"], dtype=np.float32))
    L = np.ascontiguousarray(np.asarray(inputs["L"], dtype=np.float32))
    K = np.ascontiguousarray(np.asarray(inputs["K"], dtype=np.float32))
    grad_K2 = np.ascontiguousarray(np.asarray(inputs["grad_K2"], dtype=np.float32))

    ltaug = np.concatenate(
        [L.T, 1e-3 * np.eye(D, dtype=np.float32)], axis=0
    ).astype(np.float32)

    in_maps = []
    for j in range(NCORES):
        vh = np.zeros((1, VHLEN), dtype=np.float32)
        vh[0, :MG] = np.ascontiguousarray(beta[j].T).reshape(-1)
        vh[0, MG:MTOT] = alpha[j]
        vh[0, MPAD:] = 1.0
        in_maps.append({
            "g": grad_K2[j].reshape(N, MG),
            "k": K[j],
            "vh": vh,
            "lt": ltaug,
        })

    nc = _get_nc()
    res = run_bass_kernel_spmd(nc, in_maps, core_ids=list(range(NCORES)),
                               trace=trace)
    output = np.empty((N, D), dtype=np.float32)
    for j in range(NCORES):
        col = res.results[j]["o"]          # [128, 12]
        output[:, j] = col.T.reshape(-1)[:N]
    Sigma = res.results[0]["sig"]
    return (output, Sigma), res


def kernel(**inputs):
    out, _ = run(inputs)
    return out


# revision 27
# speedup vs baseline: 1.1186x; 1.1179x over previous
"""Trainium2 Bass kernel for the ADMG RKHS-DAGMA gradient contraction.

Reference computation (D=8 variables, N=1500 observations):
    output[i, j] = sum_l alpha[j, l] * K[j, i, l]
                 + sum_{a, l} beta[j, a, l] * grad_K2[j, i, l, a]     [N, D]
    Sigma = L @ L.T + 1e-6 * I                                        [D, D]

Sharding: variable-parallel over the leading d axis — core j owns K[j]
(9 MB) and grad_K2[j] (72 MB) and produces output column j. No
collectives are needed; columns are gathered on the host.

Per-core kernel: a weighted row-sum. With G = grad_K2[j] viewed as
[N, L*A] = [1500, 12000] and vrow = concat(beta[j].T.flat, alpha[j])
(13500 f32), the output column is
    out[i] = sum_m [G | K][i, m] * vrow[m].
vrow is broadcast across the 128 partitions on-chip (PE ones-matmul into
PSUM, ScalarE drains to SBUF), then every streamed [128, C] tile goes
through one fused DVE scalar_tensor_tensor (multiply + free-axis
accumulate), so each element passes the VectorEngine exactly once
(~170us) and the kernel stays DMA-bound (~82 MB/core at ~360 GB/s).

TRN2 toolchain constraints baked in here:
  - every instruction except EventSemaphore encodes exactly ONE
    semaphore wait; Tile emits inline waits, so the kernel is structured
    (private acc tiles, clock-priming reads) to never need two;
  - the raw-ISA tensor_tensor_reduce encoding crashes the exec unit on
    this runtime; InstTensorScalarPtr (scalar_tensor_tensor with
    accum_out) performs the same fused multiply+reduce;
  - Sigma's +1e-6*I rides inside the matmul via an augmented-contraction
    operand [L.T ; 1e-3*I] so no PSUM+SBUF add is needed.
"""

import numpy as np

D = 8
N = 1500
NCORES = 8
MG = D * N            # 12000: grad_K2 inner (l, a) length
MTOT = MG + N         # 13500: plus K's l axis
P = 128
NT = (N + P - 1) // P  # 12 i-tiles (last one 92 rows)
BC = 512               # broadcast chunk (one PSUM bank of f32)
NBC = (MTOT + BC - 1) // BC          # 27 chunks
MPAD = NBC * BC                      # 13824
VHLEN = MPAD + P                     # vrow (padded) + a ones-vector tail

_COMPILED = None


def _build():
    from concourse import bacc, mybir
    from concourse.tile import TileContext

    f32 = mybir.dt.float32
    nc = bacc.Bacc()

    g = nc.declare_dram_parameter("g", [N, MG], f32, isOutput=False)
    k = nc.declare_dram_parameter("k", [N, N], f32, isOutput=False)
    # One row: vrow zero-padded to MPAD, then 128 ones (the PE broadcast
    # lhsT) — a single row so both matmul operands sit at base partition 0
    # and depend on one DMA.
    vh = nc.declare_dram_parameter("vh", [1, VHLEN], f32, isOutput=False)
    # Rows 0..D-1 = L.T, rows D..2D-1 = 1e-3 * I, so that one matmul gives
    # ltaug.T @ ltaug = L @ L.T + 1e-6 * I.
    lt = nc.declare_dram_parameter("lt", [2 * D, D], f32, isOutput=False)
    o = nc.declare_dram_parameter("o", [P, NT], f32, isOutput=True)
    sig = nc.declare_dram_parameter("sig", [D, D], f32, isOutput=True)

    mult = mybir.AluOpType.mult
    add = mybir.AluOpType.add

    with TileContext(nc) as tc:
        with (
            tc.tile_pool(name="const", bufs=1) as cpool,
            tc.tile_pool(name="gdata", bufs=4) as gpool,
            tc.tile_pool(name="kdata", bufs=3) as kpool,
            tc.tile_pool(name="accs", bufs=8) as apool,
            tc.tile_pool(name="psum", bufs=4, space="PSUM") as ppool,
            tc.tile_pool(name="psig", bufs=1, space="PSUM") as pspool,
        ):
            # vrow lands in partition 0 of the broadcast destination itself;
            # the per-chunk PE matmul + ScalarE drain then overwrite
            # partitions 0..127 with the replicated row (value-identical on
            # partition 0). The ones lhsT lives in the untouched tail.
            vb_sb = cpool.tile([P, VHLEN], f32)
            nc.sync.dma_start(out=vb_sb[0:1, :], in_=vh[:, :])
            for c in range(NBC):
                ps = ppool.tile([P, BC], f32, tag="bc")
                nc.tensor.matmul(ps[:, :], vb_sb[0:1, MPAD:MPAD + P],
                                 vb_sb[0:1, c * BC:(c + 1) * BC],
                                 start=True, stop=True)
                nc.scalar.copy(out=vb_sb[:, c * BC:(c + 1) * BC], in_=ps[:, :])

            # Primers (see loop below) stage DVE's vector clock against the
            # ScalarE broadcast copies: each tensor_copy waits on the copy
            # covering the end of a vb range, after which the STTs reading
            # that range carry only their own DMA wait. Each primer gets a
            # private slot — sharing one tile would chain same-engine WAW
            # deps and push an instruction to two waits.
            def primer_read(col):
                pt = apool.tile([P, 2], f32, tag="acc")
                nc.vector.tensor_copy(out=pt[:1, 0:1],
                                      in_=vb_sb[:1, col:col + 1])

            out_all = cpool.tile([P, NT], f32)
            nc.vector.memset(out_all[:, :], 0.0)

            GC = MG // 2   # 6000-wide g chunks (two per i-tile) to fit SBUF
            for t in range(NT):
                p = min(P, N - t * P)
                i0 = t * P
                # Fused multiply+reduce per streamed tile: out = (data * 1.0)
                # * vb, accum = row-sum. Column 1 of each private acc tile
                # absorbs the dead full-size out via a stride-0 broadcast AP.
                accs = []
                for c in range(2):
                    gt = gpool.tile([P, GC], f32, tag="g")
                    nc.sync.dma_start(out=gt[:p, :],
                                      in_=g[i0:i0 + p, c * GC:(c + 1) * GC])
                    if t == 0:
                        primer_read((c + 1) * GC - 1)
                    acc = apool.tile([P, 2], f32, tag="acc")
                    nc.vector.scalar_tensor_tensor(
                        out=acc[:p, 1:2].broadcast_to((p, GC)),
                        in0=gt[:p, :],
                        scalar=1.0,
                        in1=vb_sb[:p, c * GC:(c + 1) * GC],
                        op0=mult,
                        op1=mult,
                        accum_out=acc[:p, 0:1],
                    )
                    accs.append(acc)
                kt = kpool.tile([P, N], f32, tag="k")
                nc.sync.dma_start(out=kt[:p, :], in_=k[i0:i0 + p, :])
                if t == 0:
                    primer_read(MPAD - 1)
                acc = apool.tile([P, 2], f32, tag="acc")
                nc.vector.scalar_tensor_tensor(
                    out=acc[:p, 1:2].broadcast_to((p, N)),
                    in0=kt[:p, :],
                    scalar=1.0,
                    in1=vb_sb[:p, MG:MTOT],
                    op0=mult,
                    op1=mult,
                    accum_out=acc[:p, 0:1],
                )
                accs.append(acc)
                nc.vector.scalar_tensor_tensor(
                    out=out_all[:p, t:t + 1],
                    in0=accs[0][:p, 0:1],
                    scalar=accs[1][:p, 0:1],
                    in1=accs[2][:p, 0:1],
                    op0=add,
                    op1=add,
                )
            nc.sync.dma_start(out=o[:, :], in_=out_all[:, :])

            # Sigma = ltaug.T @ ltaug on the TensorEngine. Emitted after the
            # loop so its DVE PSUM-drain doesn't stall the streaming STTs at
            # kernel start.
            lt_sb = cpool.tile([2 * D, D], f32)
            nc.sync.dma_start(out=lt_sb[:, :], in_=lt[:, :])
            sig_ps = pspool.tile([D, D], f32)
            nc.tensor.matmul(sig_ps[:, :], lt_sb[:, :], lt_sb[:, :],
                             start=True, stop=True)
            sig_sb = cpool.tile([D, D], f32)
            nc.vector.tensor_copy(out=sig_sb[:, :], in_=sig_ps[:, :])
            nc.sync.dma_start(out=sig[:, :], in_=sig_sb[:, :])
    nc.finalize()
    return nc


def _get_nc():
    global _COMPILED
    if _COMPILED is None:
        _COMPILED = _build()
    return _COMPILED


def run(inputs, trace=False):
    """Run the SPMD kernel; returns ((output, Sigma), BassKernelResults)."""
    from concourse.bass_utils import run_bass_kernel_spmd

    alpha = np.ascontiguousarray(np.asarray(inputs["alpha"], dtype=np.float32))
    beta = np.ascontiguousarray(np.asarray(inputs["beta"], dtype=np.float32))
    L = np.ascontiguousarray(np.asarray(inputs["L"], dtype=np.float32))
    K = np.ascontiguousarray(np.asarray(inputs["K"], dtype=np.float32))
    grad_K2 = np.ascontiguousarray(np.asarray(inputs["grad_K2"], dtype=np.float32))

    ltaug = np.concatenate(
        [L.T, 1e-3 * np.eye(D, dtype=np.float32)], axis=0
    ).astype(np.float32)

    in_maps = []
    for j in range(NCORES):
        vh = np.zeros((1, VHLEN), dtype=np.float32)
        vh[0, :MG] = np.ascontiguousarray(beta[j].T).reshape(-1)
        vh[0, MG:MTOT] = alpha[j]
        vh[0, MPAD:] = 1.0
        in_maps.append({
            "g": grad_K2[j].reshape(N, MG),
            "k": K[j],
            "vh": vh,
            "lt": ltaug,
        })

    nc = _get_nc()
    res = run_bass_kernel_spmd(nc, in_maps, core_ids=list(range(NCORES)),
                               trace=trace)
    output = np.empty((N, D), dtype=np.float32)
    for j in range(NCORES):
        col = res.results[j]["o"]          # [128, 12]
        output[:, j] = col.T.reshape(-1)[:N]
    Sigma = res.results[0]["sig"]
    return (output, Sigma), res


def kernel(**inputs):
    out, _ = run(inputs)
    return out
